# revision 2
# baseline (speedup 1.0000x reference)
"""Causal self-attention (B=4, T=2048, C=1024, H=16) on 8 TRN2 NeuronCores, v3.

Sharding as v1 (core c -> batch c//2, heads [8*(c%2), 8*(c%2)+8)).  Two main
changes vs the fp32r baseline:

1. All PE matmuls run on bf16 operands.  fp32r matmuls execute at 2
   cycles/row on TRN2 (two-pass fp32 emulation); bf16 runs at 1 - this
   halves Tensor-engine time for qkv/scores/PV/proj.  Inputs are converted
   to bf16 on the host (also halves input DMA).  Accumulation stays fp32
   in PSUM.  Measured numerics: ~4e-3 vs the 2e-2 gate.
2. exp() is split between the ACT engine (native Exp -> bf16) and the DVE
   (Schraudolph bit-trick: int16(A*x+B) bit-viewed as bf16, validated on
   HW at ~7.5e-3), so the softmax no longer serializes behind a single
   engine.  PSUM evacuations (qkT/v/proj-out/yraw) run as ACT
   activation-Copies; the DVE keeps reciprocal + normalize-mul.
"""
from contextlib import ExitStack

import numpy as np
import concourse.bass as bass
import concourse.mybir as mybir
import concourse.tile as tile
from concourse import bacc

F32 = mybir.dt.float32
F32R = mybir.dt.float32r
I16 = mybir.dt.int16
BF16 = mybir.dt.bfloat16
EXP = mybir.ActivationFunctionType.Exp
COPY = mybir.ActivationFunctionType.Copy
MULT = mybir.AluOpType.mult
ADD = mybir.AluOpType.add

T = 2048          # tokens
C = 1024          # channels
NH = 8            # local heads
HD = 64           # head dim
CL = NH * HD      # local channels (512)
TJ = T // 512     # 4 q-chunks of 512
KC = T // 128     # 16 k-chunks of 128
SCALE = HD ** -0.5
MASKV = -600.0    # pre-scale mask addend: exp-arg -75, inside bit-trick range

# Schraudolph exp in bf16 bit-space: exp(x*SCALE) ~ bitcast_bf16(int16(EXA*x+EXB))
EXA = float(np.float32((2.0 ** 7) / np.log(2.0) * SCALE))
EXB = float(np.float32((127.0 - 0.0435) * 2.0 ** 7 + 0.5))

# engine split: DVE takes diagonal tiles except every `diag_act_mod`-th
# (0 = all diag on DVE), plus every `dve_extra`-th off-diagonal tile.
CFG = dict(diag_act_mod=0, dve_extra=24)


def build_nc(loop_reps: int | None = None, cfg: dict | None = None):
    cfg = {**CFG, **(cfg or {})}
    nc = bacc.Bacc("TRN2", target_bir_lowering=False, debug=False)
    xT = nc.declare_dram_parameter("xT", [C, T], BF16, isOutput=False)
    wqk = nc.declare_dram_parameter("wqk", [C, 2 * CL], BF16, isOutput=False)
    wv = nc.declare_dram_parameter("wv", [C, CL], BF16, isOutput=False)
    wp = nc.declare_dram_parameter("wp", [4, 128, C], BF16, isOutput=False)
    idn = nc.declare_dram_parameter("idn", [128, 128], BF16, isOutput=False)
    maskm = nc.declare_dram_parameter("maskm", [128, 128], BF16, isOutput=False)
    yout = nc.declare_dram_parameter("yout", [T, C], F32, isOutput=True)

    with ExitStack() as ctx:
        ctx.enter_context(nc.allow_low_precision(
            reason="bf16 PE operands; fp32 PSUM accumulation; bit-trick exp"))
        tc = ctx.enter_context(tile.TileContext(nc, pool_alloc_mode="queue"))

        # ---- persistent pools ----
        consts = ctx.enter_context(tc.tile_pool(name="consts", bufs=1))
        idn_sb = consts.tile([128, 128], BF16)
        maskm_sb = consts.tile([128, 128], BF16)
        ones_sb = consts.tile([65, 64], BF16)
        nc.sync.dma_start(idn_sb[:], idn[:])
        nc.sync.dma_start(maskm_sb[:], maskm[:])
        nc.gpsimd.memset(ones_sb[64:65, :], 1.0)

        qk_pool = ctx.enter_context(tc.tile_pool(name="qk_pool", bufs=1))
        qkT = [qk_pool.tile([128, T], BF16, name=f"qkT{fi}") for fi in range(8)]
        vaug_pool = ctx.enter_context(tc.tile_pool(name="vaug_pool", bufs=1))
        vaug = [vaug_pool.tile([128, NH * 65], BF16, name=f"vaug{tt}")
                for tt in range(KC)]
        # ones columns written once (never touched by the v copies)
        for tt in range(KC):
            va = vaug[tt].rearrange("p (h s) -> p h s", s=65)
            nc.gpsimd.memset(va[:, :, 64:65], 1.0)
        wp_pool = ctx.enter_context(tc.tile_pool(name="wp_pool", bufs=1))
        wp_sb = [wp_pool.tile([128, C], BF16, name=f"wp{pp}") for pp in range(4)]
        for pp in range(4):
            nc.sync.dma_start(wp_sb[pp][:], wp[pp, :, :])

        loop = tc.For_i(0, loop_reps) if loop_reps is not None else None
        if loop is not None:
            ctx.enter_context(loop)

        # =========== phase 1: qkv projections ===========
        with tc.tile_pool(name="w1", bufs=1) as w1, \
             tc.tile_pool(name="xp", bufs=12) as xp, \
             tc.tile_pool(name="ps1", bufs=6, space="PSUM") as ps1:
            wqk_sb = [w1.tile([128, 2 * CL], BF16, name=f"wqk{ci}") for ci in range(8)]
            wv_sb = [w1.tile([128, CL], BF16, name=f"wv{ci}") for ci in range(8)]
            for ci in range(8):
                nc.sync.dma_start(wqk_sb[ci][:], wqk[ci * 128:(ci + 1) * 128, :])
                nc.sync.dma_start(wv_sb[ci][:], wv[ci * 128:(ci + 1) * 128, :])

            for tj in range(TJ):
                xt = []
                for ci in range(8):
                    t_ = xp.tile([128, 512], BF16, name="xt", tag="xt")
                    nc.sync.dma_start(t_[:], xT[ci * 128:(ci + 1) * 128,
                                                 tj * 512:(tj + 1) * 512])
                    xt.append(t_)
                # q,k features: out [feat 128, tok 512]
                for fi in range(8):
                    ps = ps1.tile([128, 512], F32, name="qkps", tag="qkps")
                    for ci in range(8):
                        nc.tensor.matmul(
                            ps[:],
                            (wqk_sb[ci][:, fi * 128:(fi + 1) * 128]),
                            (xt[ci][:]),
                            start=(ci == 0), stop=(ci == 7))
                    nc.scalar.activation(qkT[fi][:, tj * 512:(tj + 1) * 512],
                                         ps[:], COPY)
                # v: out [tok 128, vfeat 512] -> vaug strided (65-col groups)
                for ts in range(4):
                    tt = tj * 4 + ts
                    ps = ps1.tile([128, 512], F32, name="vps", tag="qkps")
                    for ci in range(8):
                        nc.tensor.matmul(
                            ps[:],
                            (xt[ci][:, ts * 128:(ts + 1) * 128]),
                            (wv_sb[ci][:]),
                            start=(ci == 0), stop=(ci == 7))
                    va = vaug[tt].rearrange("p (h s) -> p h s", s=65)
                    nc.scalar.activation(
                        va[:, :, 0:64],
                        ps.rearrange("p (h s) -> p h s", s=64), COPY)

        # =========== phase 2: attention + proj ===========
        ndiag = 0
        noff = 0
        with tc.tile_pool(name="apool", bufs=5) as apool, \
             tc.tile_pool(name="ipool", bufs=5) as ipool, \
             tc.tile_pool(name="ysbp", bufs=8) as ysbp, \
             tc.tile_pool(name="recp", bufs=3) as recp, \
             tc.tile_pool(name="osb", bufs=3) as osbp, \
             tc.tile_pool(name="sps", bufs=2, space="PSUM") as sps, \
             tc.tile_pool(name="yps", bufs=2, space="PSUM") as yps, \
             tc.tile_pool(name="bps", bufs=1, space="PSUM") as bps, \
             tc.tile_pool(name="ps3", bufs=1, space="PSUM") as ps3:
            for j in range(TJ):
                yts = []
                for p in range(4):
                    pair = (2 * p, 2 * p + 1)
                    # pair-stacked normalized y: head 2p -> rows 0-63 (direct
                    # DVE write), head 2p+1 -> rows 64-127 (via SBUF-SBUF DMA
                    # partition remap) so proj runs full-K=128 matmuls.
                    yt = ysbp.tile([128, 512], BF16, name="yt", tag="yt")
                    yts.append(yt)
                    att = {}
                    # ---- scores (transposed): sT[k, q] + mask + exp ----
                    # K=64 head-pair matmuls are interleaved h0,h1,h0,h1 so
                    # the two heads run concurrently in disjoint PE row groups
                    # (h even -> rows 0-63, h odd -> rows 64-127).
                    for kcg in range(2 * (j + 1)):
                        sp = {h: sps.tile([128, 1024], F32, name="sp", tag="sp")
                              for h in pair}
                        for u in range(2):
                            kc = 2 * kcg + u
                            d = max(0, (kc - 4 * j) * 128)
                            for h in pair:
                                base = (h % 2) * 64
                                ksl = qkT[4 + h // 2][base:base + 64,
                                                     kc * 128:(kc + 1) * 128]
                                qsl = qkT[h // 2][base:base + 64,
                                                  j * 512 + d:(j + 1) * 512]
                                nc.tensor.matmul(
                                    sp[h][:, u * 512 + d:(u + 1) * 512],
                                    (ksl), (qsl),
                                    start=True, stop=(kc < 4 * j),
                                    skip_group_check=True)
                            if kc >= 4 * j:  # diagonal blocks: add mask
                                for h in pair:
                                    nc.tensor.matmul(
                                        sp[h][:, u * 512 + d:u * 512 + d + 128],
                                        idn_sb[:], maskm_sb[:],
                                        start=False, stop=True,
                                        skip_group_check=True)
                        diag = kcg >= 2 * j
                        for h in pair:
                            if diag:
                                ndiag += 1
                                m = cfg["diag_act_mod"]
                                use_dve = not (m and ndiag % m == 0)
                            else:
                                noff += 1
                                m = cfg["dve_extra"]
                                use_dve = bool(m) and noff % m == 0
                            if use_dve:
                                at = ipool.tile([128, 1024], I16, name="ati",
                                                tag="ati")
                                nc.vector.tensor_scalar(
                                    at[:], sp[h][:], EXA, EXB, MULT, ADD)
                                att[(h, kcg)] = at[:].bitcast(BF16)
                            else:
                                at = apool.tile([128, 1024], BF16, name="at",
                                                tag="at")
                                nc.scalar.activation(at[:], sp[h][:], EXP,
                                                     scale=SCALE)
                                att[(h, kcg)] = at[:]
                    # ---- PV (+denominator via ones column) ----
                    for h in pair:
                        yp = yps.tile([128, 512], F32, name="yp", tag="yp")
                        for kcg in range(2 * (j + 1)):
                            for u in range(2):
                                kc = 2 * kcg + u
                                d = max(0, (kc - 4 * j) * 128)
                                nc.tensor.matmul(
                                    yp[0:65, d:512],
                                    (vaug[kc][:, h * 65:h * 65 + 65]),
                                    (att[(h, kcg)][:, u * 512 + d:(u + 1) * 512]),
                                    start=(kc == 0), stop=(kc == 4 * j + 3),
                                    skip_group_check=True)
                        # ---- normalize: reciprocal of the denominator row,
                        # PE-broadcast it across 64 partitions, scale y ----
                        rc = recp.tile([65, 512], BF16, name="rc", tag="rc")
                        nc.vector.reciprocal(rc[64:65, :], yp[64:65, :])
                        bp = bps.tile([64, 512], F32, name="bp", tag="bp")
                        nc.tensor.matmul(
                            bp[:], (ones_sb[64:65, 0:64]), (rc[64:65, :]),
                            start=True, stop=True)
                        yraw = recp.tile([64, 512], F32, name="yraw", tag="yraw")
                        nc.scalar.activation(yraw[:], yp[0:64, :], COPY)
                        if h % 2 == 0:
                            nc.vector.tensor_mul(yt[0:64, :], yraw[:], bp[:])
                        else:
                            ytmp = recp.tile([64, 512], BF16, name="ytmp",
                                             tag="ytmp")
                            nc.vector.tensor_mul(ytmp[:], yraw[:], bp[:])
                            nc.sync.dma_start(yt[64:128, :], ytmp[:])
                # ---- proj for this token block ----
                for ts in range(4):
                    for co in range(2):
                        ps = ps3.tile([128, 512], F32, name="pps", tag="pps")
                        for pp in range(4):
                            nc.tensor.matmul(
                                ps[:],
                                (yts[pp][:, ts * 128:(ts + 1) * 128]),
                                (wp_sb[pp][:, co * 512:(co + 1) * 512]),
                                start=(pp == 0), stop=(pp == 3))
                        ot = osbp.tile([128, 512], F32, name="ot", tag="ot")
                        nc.scalar.activation(ot[:], ps[:], COPY)
                        nc.sync.dma_start(
                            yout[(j * 4 + ts) * 128:(j * 4 + ts + 1) * 128,
                                 co * 512:(co + 1) * 512], ot[:])

    nc.compile()
    return nc


# ---------------- host-side sharding ----------------

def shard_inputs(x, w_qkv, w_proj):
    """Full inputs -> list of 8 per-core input maps (bf16 on host)."""
    import ml_dtypes
    BF = ml_dtypes.bfloat16
    idn = np.eye(128, dtype=BF)
    r = np.arange(128)
    maskm = np.where(r[:, None] > r[None, :], MASKV, 0.0).astype(BF)
    xb = np.asarray(x, np.float32).astype(BF)
    wb = np.asarray(w_qkv, np.float32).astype(BF)
    wpb = np.asarray(w_proj, np.float32).astype(BF)
    in_maps = []
    for core in range(8):
        b, g = core // 2, core % 2
        sl = slice(g * CL, (g + 1) * CL)
        in_maps.append(dict(
            xT=np.ascontiguousarray(xb[b].T),
            wqk=np.ascontiguousarray(
                np.concatenate([wb[:, sl], wb[:, C + g * CL:C + (g + 1) * CL]],
                               axis=1)),
            wv=np.ascontiguousarray(wb[:, 2 * C + g * CL:2 * C + (g + 1) * CL]),
            wp=np.ascontiguousarray(wpb[sl, :].reshape(4, 128, C)),
            idn=idn, maskm=maskm,
        ))
    return in_maps


def unshard_output(results, b_proj):
    """Per-core partial [T, C] projections -> full [B, T, C] output."""
    out = np.empty((4, T, C), dtype=np.float32)
    for b in range(4):
        out[b] = results[2 * b]["yout"] + results[2 * b + 1]["yout"]
    out += b_proj[None, None, :]
    return out


_CACHE = {}


def kernel(x, w_qkv, w_proj, b_proj):
    from concourse.bass_utils import run_bass_kernel_spmd
    if "nc" not in _CACHE:
        _CACHE["nc"] = build_nc()
    nc = _CACHE["nc"]
    in_maps = shard_inputs(np.asarray(x, np.float32),
                           np.asarray(w_qkv, np.float32),
                           np.asarray(w_proj, np.float32))
    res = run_bass_kernel_spmd(nc, in_maps, core_ids=list(range(8)))
    return unshard_output(res.results, np.asarray(b_proj, np.float32))
